# revision 14
# baseline (speedup 1.0000x reference)
"""Trainium2 Bass kernel for nn_STSourceModule (segment_reduce).

Math: source_ids x are binary {0,1}, so the masked softmax over sites
collapses to a closed form.  With g[n] = exp(fire_bias[n]),
A0[h] = exp(attn_b[h]), A1[h] = exp(attn_b[h] + attn_w[h]):

  Z[s,h,c]   = A0[h]*(S0[c] - T1[s,c]) + A1[h]*T1[s,c]
  r[s,c,h]   = A1[h]*T1[s,c] / Z[s,h,c]
  out[s,c,:] = mask[c]*base + sum_h r[s,c,h]*Wh[h,:]

where S0[c] = sum_{n in c} g[n], T1[s,c] = sum_{n in c} x[s,n]*g[n],
base = ffn_b + ffn_w@val_b, Wh[h] = ffn_w[:,32h:32h+32]@val_w[32h:32h+32].

Sharding: data-parallel over batch B=8, one batch element per core.

Expansion trick: R@W in fp16-pair precision as a SINGLE K=20 matmul per
tile — lhsT stacks [Rhi;Rlo;Rhi;Rlo] (fp16) and rhs stacks
[Wh;Wl;Wl;Wh], so the K-contraction sums all four partial products
(Rhi@Wh + Rlo@Wl + Rhi@Wl + Rlo@Wh = exact pair product) in fp32 PSUM
at the PE cost of one 256-column fp16 pass (~214ns/tile-pass).
"""

import sys

for _p in ("/opt/trn_rl_repo",):
    if _p not in sys.path:
        sys.path.insert(0, _p)

from contextlib import ExitStack

import numpy as np

import concourse.bass as bass
import concourse.tile as tile
from concourse import bacc, mybir
from concourse.bass_utils import run_bass_kernel_spmd
from concourse.masks import make_identity

F32 = mybir.dt.float32
F16 = mybir.dt.float16
AF = mybir.ActivationFunctionType
ALU = mybir.AluOpType

MAX_SP, MAX_TP = 180.0, 365.0
B, S, N, C = 8, 256, 128, 64
NH, HID, FH = 4, 256, 32

TRACE = False           # set True (e.g. from test.py) to neuron-profile
LAST_RESULT = None      # BassKernelResults of the last run


def _build_program(csp, ctp, a0, a1):
    nc = bacc.Bacc()

    x_d = nc.declare_dram_parameter("x", [S, N], F32, isOutput=False)
    nv_d = nc.declare_dram_parameter("nv", [N, 4], F32, isOutput=False)
    cv_d = nc.declare_dram_parameter("cv", [1, 192], F32, isOutput=False)
    wq_d = nc.declare_dram_parameter("wq", [20, HID], F16, isOutput=False)
    out_d = nc.declare_dram_parameter("out", [S, C, HID], F32, isOutput=True)

    da = [float(a1[h] - a0[h]) for h in range(NH)]

    with tile.TileContext(nc) as tc, ExitStack() as ctx:
        consts = ctx.enter_context(tc.tile_pool(name="consts", bufs=1))
        work = ctx.enter_context(tc.tile_pool(name="work", bufs=2))
        tpsum = ctx.enter_context(tc.tile_pool(name="tpsum", bufs=2, space="PSUM"))
        opsum = ctx.enter_context(tc.tile_pool(name="opsum", bufs=5, space="PSUM"))
        outp = ctx.enter_context(tc.tile_pool(name="outp", bufs=4))

        # ---- inputs (one DMA each) -------------------------------------
        ident = consts.tile([128, 128], F32)
        make_identity(nc, ident)

        cv = consts.tile([128, 192], F32)
        nc.sync.dma_start(out=cv, in_=cv_d[:, :].to_broadcast([128, 192]))
        wq = consts.tile([20, HID], F16)
        nc.sync.dma_start(out=wq, in_=wq_d[:, :])
        nv = consts.tile([128, 4], F32)
        nc.sync.dma_start(out=nv, in_=nv_d[:, :])
        xf = consts.tile([128, 2, N], F32)
        nc.sync.dma_start(out=xf, in_=x_d[:, :].rearrange("(q p) n -> p q n", p=128))

        ones_col = consts.tile([128, 1], F32)
        nc.vector.memset(ones_col, 1.0)

        # cv layout: [iota(64) | w1s(32) w1t(32) | w2s(32) w2t(32)]
        iot, w1cat, w2cat = cv[:, 0:64], cv[:, 64:128], cv[:, 128:192]
        labs, dsp, dtp = nv[:, 0:1], nv[:, 1:2], nv[:, 2:3]

        # ---- FIRE bias -> g = exp(bias) --------------------------------
        dls = work.tile([128, 1], F32)
        nc.scalar.activation(out=dls, in_=dsp, func=AF.Ln, bias=1.0, scale=csp)
        dlt = work.tile([128, 1], F32)
        nc.scalar.activation(out=dlt, in_=dtp, func=AF.Ln, bias=1.0, scale=ctp)
        h = work.tile([128, 2 * FH], F32)
        nc.vector.tensor_scalar_mul(out=h[:, 0:FH], in0=w1cat[:, 0:FH], scalar1=dls)
        nc.vector.tensor_scalar_mul(out=h[:, FH:], in0=w1cat[:, FH:], scalar1=dlt)
        nc.scalar.activation(out=h, in_=h, func=AF.Silu)
        nc.vector.tensor_mul(out=h, in0=h, in1=w2cat)
        bsum = work.tile([128, 1], F32)
        nc.vector.reduce_sum(out=bsum, in_=h, axis=mybir.AxisListType.X)
        g = work.tile([128, 1], F32)
        nc.scalar.activation(out=g, in_=bsum, func=AF.Exp)

        # ---- mg[n,c] = (lab[n]==c) * g[n] ------------------------------
        mg = work.tile([128, C], F32)
        nc.vector.tensor_scalar(
            out=mg, in0=iot, scalar1=labs, scalar2=g,
            op0=ALU.is_equal, op1=ALU.mult,
        )

        # ---- xT via PE transpose (dummy first: lone-LDWEIGHTS sync) ----
        ptd = tpsum.tile([128, 128], F32, tag="pt")
        nc.tensor.transpose(ptd, ident, ident)
        xT = work.tile([128, S], F32)
        for i in range(2):
            pt = tpsum.tile([128, 128], F32, tag="pt")
            nc.tensor.transpose(pt, xf[:, i, :], ident)
            nc.scalar.copy(out=xT[:, i * 128:(i + 1) * 128], in_=pt)

        # ---- segment sums ----------------------------------------------
        t1_ps = tpsum.tile([64, S], F32, tag="pt")
        nc.tensor.matmul(t1_ps, lhsT=mg, rhs=xT, start=True, stop=True)

        s0_ps = tpsum.tile([64, 1], F32, tag="pt")
        nc.tensor.matmul(s0_ps, lhsT=mg, rhs=ones_col, start=True, stop=True)

        mask = work.tile([64, 1], F32)
        nc.vector.tensor_scalar(out=mask, in0=s0_ps, scalar1=0.0, scalar2=None,
                                op0=ALU.is_gt)
        maskc = work.tile([64, 1], F32)
        nc.vector.tensor_scalar(out=maskc, in0=s0_ps, scalar1=0.0, scalar2=None,
                                op0=ALU.is_le)
        sams = work.tile([64, NH], F32)
        for hh in range(NH):
            # A0*S0 + (1-mask): empty clusters get denominator 1
            nc.vector.tensor_scalar(out=sams[:, hh:hh + 1], in0=s0_ps,
                                    scalar1=float(a0[hh]), scalar2=maskc,
                                    op0=ALU.mult, op1=ALU.add)

        # ---- r planes + fp16 hi/lo split + flatten, per s-half ---------
        # rt20 partitions: [0:5]=Rhi, [5:10]=Rlo, [10:15]=Rhi, [15:20]=Rlo
        rt20 = consts.tile([20, C, 2, S // 2], F16)
        H = S // 2

        for sh in range(2):
            sr = slice(sh * H, (sh + 1) * H)
            t1h = t1_ps[:, sr]

            def split_and_flatten(j, plane):
                hi = work.tile([64, H], F16, tag="hi16")
                nc.vector.tensor_copy(out=hi, in_=plane)
                lo = work.tile([64, H], F16, tag="lo16")
                nc.vector.tensor_sub(out=lo, in0=plane, in1=hi)
                nc.sync.dma_start(out=rt20[j:j + 1, :, sh, :], in_=hi)
                nc.sync.dma_start(out=rt20[10 + j:11 + j, :, sh, :], in_=hi)
                nc.sync.dma_start(out=rt20[5 + j:6 + j, :, sh, :], in_=lo)
                nc.sync.dma_start(out=rt20[15 + j:16 + j, :, sh, :], in_=lo)

            mpl = work.tile([64, H], F32, tag="mpl")
            nc.vector.tensor_scalar(out=mpl, in0=t1h, scalar1=0.0,
                                    scalar2=mask, op0=ALU.mult, op1=ALU.add)
            split_and_flatten(0, mpl)
            rscr = work.tile([64, H], F32, tag="rscr")
            for hh in range(NH):
                den = work.tile([64, H], F32, tag="den")
                nc.vector.tensor_scalar(out=den, in0=t1h, scalar1=da[hh],
                                        scalar2=sams[:, hh:hh + 1],
                                        op0=ALU.mult, op1=ALU.add)
                rinv = work.tile([64, H], F32, tag="rinv")
                nc.vector.reciprocal_approx_accurate(out=rinv, in_=den,
                                                     scratch=rscr)
                rpl = work.tile([64, H], F32, tag="rpl")
                nc.vector.scalar_tensor_tensor(
                    out=rpl, in0=t1h, scalar=float(a1[hh]), in1=rinv,
                    op0=ALU.mult, op1=ALU.mult,
                )
                split_and_flatten(1 + hh, rpl)

            # ---- expansion for this s-half: one K=20 matmul per c ------
            for cp in range(C // 2):
                ps = opsum.tile([128, 2, HID], F32, tag="ops")  # [p, ci, k]
                for ci in range(2):
                    nc.tensor.matmul(ps[:, ci, :],
                                     lhsT=rt20[:, cp * 2 + ci, sh, :],
                                     rhs=wq, start=True, stop=True)
                st = outp.tile([128, 2, HID], F32, tag="st")
                nc.scalar.copy(out=st, in_=ps)
                nc.sync.dma_start(
                    out=out_d[sh * H:(sh + 1) * H, cp * 2:cp * 2 + 2, :],
                    in_=st,
                )

    nc.finalize()
    return nc


_CACHE = {}


def _program(csp, ctp, a0, a1):
    key = (csp, ctp, tuple(a0), tuple(a1))
    if key not in _CACHE:
        _CACHE[key] = _build_program(csp, ctp, a0, a1)
    return _CACHE[key]


def kernel(source_ids, source_cluster_labels, in_cluster_spatial_dist,
           in_cluster_temporal_dist, num_clusters,
           c_sp, sp_w1, sp_w2, c_tp, tp_w1, tp_w2,
           attn_w, attn_b, val_w, val_b, ffn_w, ffn_b):
    global LAST_RESULT

    x = np.ascontiguousarray(np.asarray(source_ids), dtype=np.float32)
    lab = np.asarray(source_cluster_labels).astype(np.float32)
    dsp = np.asarray(in_cluster_spatial_dist).astype(np.float32)
    dtp = np.asarray(in_cluster_temporal_dist).astype(np.float32)
    assert int(np.asarray(num_clusters)) == C

    csp = float(max(float(np.asarray(c_sp)), 0.0))
    ctp = float(max(float(np.asarray(c_tp)), 0.0))
    lsp = float(np.log(csp * MAX_SP + 1.0))
    ltp = float(np.log(ctp * MAX_TP + 1.0))

    sp_w1 = np.asarray(sp_w1, dtype=np.float32)   # (FH,1)
    sp_w2 = np.asarray(sp_w2, dtype=np.float32)   # (1,FH)
    tp_w1 = np.asarray(tp_w1, dtype=np.float32)
    tp_w2 = np.asarray(tp_w2, dtype=np.float32)

    cv = np.zeros((1, 192), dtype=np.float32)
    cv[0, 0:64] = np.arange(C, dtype=np.float32)
    cv[0, 64:96] = sp_w1[:, 0] / lsp
    cv[0, 96:128] = tp_w1[:, 0] / ltp
    cv[0, 128:160] = sp_w2[0]
    cv[0, 160:192] = tp_w2[0]

    attn_w = np.asarray(attn_w, dtype=np.float64)
    attn_b = np.asarray(attn_b, dtype=np.float64)
    a0 = np.exp(attn_b)
    a1 = np.exp(attn_b + attn_w)

    val_w = np.asarray(val_w, dtype=np.float64)
    val_b = np.asarray(val_b, dtype=np.float64)
    ffn_w = np.asarray(ffn_w, dtype=np.float64)
    ffn_b = np.asarray(ffn_b, dtype=np.float64)
    waug = np.zeros((5, HID), dtype=np.float64)
    waug[0] = ffn_b + ffn_w @ val_b
    for h in range(NH):
        blk = slice(h * 32, (h + 1) * 32)
        waug[1 + h] = ffn_w[:, blk] @ val_w[blk]
    w_hi = waug.astype(np.float16)
    w_lo = (waug - w_hi.astype(np.float64)).astype(np.float16)
    # rhs stack matching lhsT [Rhi;Rlo;Rhi;Rlo]:
    # Rhi@Wh + Rlo@Wl + Rhi@Wl + Rlo@Wh = exact pair product
    wquad = np.ascontiguousarray(
        np.concatenate([w_hi, w_lo, w_lo, w_hi], axis=0))

    nc = _program(csp, ctp, tuple(a0.tolist()), tuple(a1.tolist()))

    in_maps = []
    for b in range(B):
        nv = np.zeros((N, 4), dtype=np.float32)
        nv[:, 0] = lab[b]
        nv[:, 1] = dsp[b]
        nv[:, 2] = dtp[b]
        in_maps.append({"x": x[b], "nv": nv, "cv": cv, "wq": wquad})

    res = run_bass_kernel_spmd(nc, in_maps, core_ids=list(range(B)),
                               trace=TRACE)
    LAST_RESULT = res
    out = np.stack([res.results[b]["out"] for b in range(B)], axis=0)
    return out
